# revision 33
# baseline (speedup 1.0000x reference)
"""DualAttention Trainium2 kernel (fp8 DoubleRow + bf16 qk conv).

Sharding: 8 cores = 4 samples x 2 query-halves. Per core the sample image is
"rolled" by the half offset (host-side, with correct zero padding), so every
core runs the identical program on its first 2048 query positions; attention
over key positions is permutation-invariant, so convs/attention on the rolled
image give the true result for the core's half.

Precision (tolerance is 2e-2 max-abs / absmax):
  qk conv   bf16 (fp8 conv noise on scores blows the error budget)
  v conv    fp8e4 DoubleRow, weights x16 (fp8 min-normal), /16 on convert
  scores    fp8e4 DoubleRow over channel halves (K=32), q/k requantized fp8
  exp       ACT, scale 1/8, bias -2 (keeps E' = e^(s-2) inside fp8's 240)
  U = vT E  fp8e4 DoubleRow over j-tile pairs
  D         quarter 0: Pool dacc + PE colsum; quarters 1-3: ones-matmul
  fuse      bf16 (glob path feeds x straight to the output)
The e^-2 shift cancels between U and D. The SE yse gate is folded into the
glob fuse weights on-chip, so glob = x * yse is never materialized. v is
transposed by chunked xbar DMA transposes (bf16), then Pool converts to fp8.

Schedule: conv chunks and attention are software-pipelined. qk8->q_dr/k_dr
SBUF re-layout DMAs fire every other chunk so scores for quarters 0-3 start
as early as their k-tiles exist; exp runs on an otherwise-clean ACT queue
(the 64 exp instructions are the ~67us critical resource). U lags scores by
several stages; the remaining pairs run as one flat cross-quarter pipeline
with quarter finishes (colsum/ones-matmul -> reciprocal -> ones-broadcast ->
loc16) and the per-quarter fuse emitted inline.
"""

import sys

sys.path.insert(0, "/opt/trn_rl_repo")

import numpy as np
import ml_dtypes

import concourse.bass as bass
import concourse.mybir as mybir
import concourse.tile as tile
from concourse import bacc
from concourse.bass_utils import run_bass_kernel_spmd

f32 = mybir.dt.float32
f32r = mybir.dt.float32r
bf16 = mybir.dt.bfloat16
fp8 = mybir.dt.float8e4
AF = mybir.ActivationFunctionType
DR = mybir.MatmulPerfMode.DoubleRow
f8np = ml_dtypes.float8_e4m3
b16np = ml_dtypes.bfloat16

C = 256
CT = 2          # channel tiles of 128
H = W = 64
HW = H * W      # 4096
HWh = 2048      # query positions per core
NP = 16         # j-tile pairs (256 key positions each)
NQ = 4          # i quarters of 512 query positions
N_CORES = 8
WSCALE = 16.0   # host-side v-conv weight scale (fp8 subnormal avoidance)
ESHIFT = -2.0   # exp bias: E' = exp(s + ESHIFT)

_compiled = None

# (chunk, base_row) for the 8 x 512-position conv chunks; the rolled image
# has two 34-row blocks (rows 0:34 own half, 34:68 other half).
CHUNKS = [(c, c * 8 if c < 4 else 34 + (c - 4) * 8) for c in range(8)]


def _build(debug=False):
    nc = bacc.Bacc("TRN2", target_bir_lowering=False, debug=False,
                   num_devices=N_CORES)

    xp16_d = nc.declare_dram_parameter("xp16", [128, 2, 68, 66], bf16, isOutput=False)
    xp8_d = nc.declare_dram_parameter("xp8", [128, 2, 68, 66], fp8, isOutput=False)
    xq_d = nc.declare_dram_parameter("xq", [128, 2, 32, 64], bf16, isOutput=False)
    wqk_d = nc.declare_dram_parameter("wqk", [128, 9, 2, 128], bf16, isOutput=False)
    wv_d = nc.declare_dram_parameter("wv", [128, 9, 2, 2, 128], fp8, isOutput=False)
    fusg_d = nc.declare_dram_parameter("fusg", [128, 4, 128], bf16, isOutput=False)
    fusl_d = nc.declare_dram_parameter("fusl", [128, 4, 128], bf16, isOutput=False)
    smallp_d = nc.declare_dram_parameter("smallp", [128, 39], f32, isOutput=False)
    smallq_d = nc.declare_dram_parameter("smallq", [16, 257], f32, isOutput=False)
    out_d = nc.declare_dram_parameter("out", [2, 128, HWh], f32, isOutput=True)
    if debug:
        qk8_dbg = nc.declare_dram_parameter("qk8_dbg", [128, HW], f32, isOutput=True)
        vt_dbg = nc.declare_dram_parameter("vt_dbg", [128, NP * 512], f32, isOutput=True)
        d_dbg = nc.declare_dram_parameter("d_dbg", [1, HWh], f32, isOutput=True)
        loc_dbg = nc.declare_dram_parameter("loc_dbg", [128, 2, HWh], f32, isOutput=True)

    with tile.TileContext(nc) as tc, \
         nc.allow_low_precision(reason="fp8/bf16 storage; tolerance 2e-2"):
      with tc.tile_pool(name="pw", bufs=1) as pw:
        # ---- persistent tiles
        wqk = pw.tile([128, 9, 2, 128], bf16)
        wv = pw.tile([128, 9, 2, 2, 128], fp8)
        fusg = pw.tile([128, 4, 128], bf16)
        fusl = pw.tile([128, 4, 128], bf16)
        sp = pw.tile([128, 39], f32)
        sq = pw.tile([16, 257], f32)
        xp16 = pw.tile([128, 2, 68, 66], bf16)
        xp8 = pw.tile([128, 2, 68, 66], fp8)
        xq = pw.tile([128, 2, 32, 64], bf16)
        qk8 = pw.tile([128, HW], fp8)
        q_dr = pw.tile([32, 2, HWh], fp8)
        k_dr = pw.tile([32, NP, 2, 2, 128], fp8)        # (pair, sub, chanhalf, j)
        vt16 = pw.tile([128, 2, 8, 2, 2, 128], bf16)    # (ct, chunk, pl, s, c)
        vt = pw.tile([128, 2, 8, 2, 2, 128], fp8)
        vsb = [pw.tile([128, HW], bf16, tag=f"vsb{t}", name=f"vsb{t}")
               for t in range(CT)]
        loc16 = pw.tile([128, 2, HWh], bf16)            # (ct, i)
        ones8 = pw.tile([128, 2, 32], fp8)
        oner = pw.tile([1, 128], f32r)
        onerf = pw.tile([1, 128], f32)
        onecf = pw.tile([128, 1], f32)
        onec = pw.tile([128, 1], f32r)
        ebias = pw.tile([128, 1], f32)
        dr = pw.tile([1, HWh], f32r)
        dacc = [pw.tile([128, 512], f32r, tag=f"dacc{i}", name=f"dacc{i}")
                for i in range(4)]
        recb = [pw.tile([128, 512], f32, tag=f"recb{i}", name=f"recb{i}")
                for i in range(2)]
        yse = [pw.tile([128, 1], f32, tag=f"yse{t}", name=f"yse{t}")
               for t in range(CT)]
        sums = [pw.tile([128, 1], f32, tag=f"sums{t}", name=f"sums{t}")
                for t in range(CT)]

        # ---- input DMAs (first conv chunk needs wqk + xp16 rows 0:10)
        nc.sync.dma_start(wqk[:, 0:1, :, :], wqk_d[:, 0:1, :, :])
        nc.sync.dma_start(xp16[:, :, 0:6, :], xp16_d[:, :, 0:6, :])
        nc.sync.dma_start(wqk[:, 1:3, :, :], wqk_d[:, 1:3, :, :])
        nc.sync.dma_start(xp16[:, :, 6:12, :], xp16_d[:, :, 6:12, :])
        nc.sync.dma_start(wqk[:, 3:9, :, :], wqk_d[:, 3:9, :, :])
        nc.sync.dma_start(xp8[:, :, 0:12, :], xp8_d[:, :, 0:12, :])
        nc.sync.dma_start(wv[:], wv_d[:])
        nc.sync.dma_start(sp[:], smallp_d[:])
        nc.sync.dma_start(sq[:], smallq_d[:])
        for r0, r1 in [(12, 23), (23, 34), (34, 46), (46, 57), (57, 68)]:
            nc.sync.dma_start(xp16[:, :, r0:r1, :], xp16_d[:, :, r0:r1, :])
            nc.sync.dma_start(xp8[:, :, r0:r1, :], xp8_d[:, :, r0:r1, :])

        nc.vector.memset(ones8[:], 1.0)
        nc.vector.memset(onerf[:], 1.0)
        nc.vector.tensor_copy(oner[:], onerf[:])
        nc.vector.memset(onecf[:], 1.0)
        nc.vector.tensor_copy(onec[:], onecf[:])
        nc.vector.memset(ebias[:], ESHIFT)

        ets = {}

        def sc_stage(q, pr, dacc_t, psT, pet, pD=None):
            """scores -> exp (fp8 et); D via DVE dacc (q0) or pD matmul."""
            isl = slice(q * 512, (q + 1) * 512)
            pT = psT.tile([128, 2, 512], f32, tag="pT", name=f"pT{q}_{pr}")
            for s in range(2):
                nc.tensor.matmul(pT[:, s, :], k_dr[:, pr, s, :, :],
                                 q_dr[:, :, isl], start=True, stop=True,
                                 perf_mode=DR)
            et = pet.tile([128, 2, 512], fp8, tag="et", name=f"et{q}_{pr}")
            nc.scalar.activation(et[:], pT[:], AF.Exp, bias=ebias[:, 0:1],
                                 scale=0.125)
            if dacc_t is not None:
                eng = nc.gpsimd
                if pr == 0:
                    eng.tensor_copy(dacc_t[:], et[:, 0, :])
                else:
                    eng.tensor_add(dacc_t[:], dacc_t[:], et[:, 0, :])
                eng.tensor_add(dacc_t[:], dacc_t[:], et[:, 1, :])
            ets[(q, pr)] = (et, pD)

        def u_stage(q, pr, pu, pD=None):
            et, _ = ets.pop((q, pr))
            for t in range(CT):
                nc.tensor.matmul(pu[t][:], vt[:, t, pr // 2, pr % 2, :, :],
                                 et[:], start=(pr == 0), stop=(pr == NP - 1),
                                 perf_mode=DR)
            if pD is not None:
                nc.tensor.matmul(pD[:], ones8[:], et[:],
                                 start=(pr == 0), stop=(pr == NP - 1),
                                 perf_mode=DR)

        def finish_quarter(q, pu, psD, dacc_t, pD=None):
            isl = slice(q * 512, (q + 1) * 512)
            if dacc_t is not None:
                pD = psD.tile([32, 512], f32, tag="pB", name=f"pDc{q}")
                nc.tensor.matmul(pD[0:1, :], onec[:], dacc_t[:],
                                 start=True, stop=True)
            nc.vector.reciprocal(dr[:, isl], pD[0:1, :])
            pB = psD.tile([128, 512], f32, tag="pB", name=f"pB{q}")
            nc.tensor.matmul(pB[:], oner[:], dr[:, isl], start=True, stop=True)
            rb = recb[q % 2]
            nc.vector.tensor_copy(rb[:], pB[:])
            for t in range(CT):
                nc.vector.tensor_mul(loc16[:, t, isl], pu[t][:], rb[:])

        po = None

        def emit_fuse_quarter(ic):
            isl = slice(ic * 512, (ic + 1) * 512)
            for mt in range(CT):
                pf = psD.tile([128, 512], f32, tag="pB", name=f"pf{ic}_{mt}")
                nc.tensor.matmul(pf[:], fusl[:, 0 + mt, :], loc16[:, 0, isl],
                                 start=True, stop=False)
                nc.tensor.matmul(pf[:], fusl[:, 2 + mt, :], loc16[:, 1, isl],
                                 start=False, stop=False)
                nc.tensor.matmul(pf[:], fusg[:, 0 + mt, :],
                                 xq[:, 0, ic * 8:(ic + 1) * 8, :],
                                 start=False, stop=False)
                nc.tensor.matmul(pf[:], fusg[:, 2 + mt, :],
                                 xq[:, 1, ic * 8:(ic + 1) * 8, :],
                                 start=False, stop=True)
                ob = po.tile([128, 512], f32, tag="ob", name=f"ob{ic}_{mt}")
                nc.vector.tensor_scalar_add(ob[:], pf[:], sp[:, 1 + mt:2 + mt])
                nc.sync.dma_start(out_d[mt, :, isl], ob[:])

        with tc.tile_pool(name="pet", bufs=24) as pet, \
             tc.tile_pool(name="psT", bufs=2, space="PSUM") as psT, \
             tc.tile_pool(name="psU", bufs=1, space="PSUM") as psU:
            pu0 = [psU.tile([128, 512], f32, tag=f"pu{t}", name=f"pu{t}_0")
                   for t in range(CT)]
            def emit_se(psSE):
                def se_psum(name):
                    t = psSE.tile([128, 2, 512], f32, tag="pT", name=name)
                    return t
                # ---- SE channel sums on ACT (accum_out); scratch fp8 output
                sa = pw.tile([128, 1], f32)
                sb_ = pw.tile([128, 1], f32)
                scr = pw.tile([128, 32, 64], fp8, tag="sescr")
                for j in range(CT):
                    nc.scalar.activation(scr[:], xp8[:, j, 1:33, 1:65],
                                         AF.Copy, accum_out=sa[:])
                    nc.scalar.activation(scr[:], xp8[:, j, 35:67, 1:65],
                                         AF.Copy, accum_out=sb_[:])
                    nc.vector.tensor_add(sums[j][:], sa[:], sb_[:])
                # ---- SE MLP: yse = sigmoid(fc2 @ relu(fc1 @ mean + b1) + b2)
                ps1 = se_psum("ps1")
                for j in range(CT):
                    nc.tensor.matmul(ps1[0:16, 0, 0:1],
                                     sp[:, 7 + j * 16:7 + (j + 1) * 16],
                                     sums[j][:], start=(j == 0),
                                     stop=(j == CT - 1))
                y1 = pw.tile([16, 1], f32)
                nc.scalar.activation(y1[:], ps1[0:16, 0, 0:1], AF.Relu,
                                     bias=sq[0:16, 256:257])
                for t in range(CT):
                    ps2 = se_psum(f"ps2_{t}")
                    nc.tensor.matmul(ps2[:, 0, 0:1],
                                     sq[0:16, t * 128:(t + 1) * 128],
                                     y1[:], start=True, stop=True)
                    # sigmoid(z) = 1/(1+exp(-z)); sp col 5+t holds -fc2_b
                    en = pw.tile([128, 1], f32, tag="en")
                    nc.scalar.activation(en[:], ps2[:, 0, 0:1], AF.Exp,
                                         bias=sp[:, 5 + t:6 + t], scale=-1.0)
                    nc.vector.tensor_scalar_add(en[:], en[:], 1.0)
                    nc.vector.reciprocal(yse[t][:], en[:])
            with tc.tile_pool(name="psC", bufs=2, space="PSUM") as psC:
                # ---- conv chunks; quarter-0 attention software-pipelined in
                # (scores lag 2 chunks, U lags 3, so PE never waits on the
                # qk8->k_dr DMA chain or on ACT's exp)
                for c, base in CHUNKS:
                    csl = slice(c * 512, (c + 1) * 512)
                    # qk conv (bf16, 18 passes)
                    pqk = psC.tile([128, 512], f32, tag="pc", name=f"pqk{c}")
                    for t in range(9):
                        for j in range(CT):
                            nc.tensor.matmul(
                                pqk[:], wqk[:, t, j, :],
                                xp16[:, j, base + t // 3:base + t // 3 + 8,
                                     t % 3:t % 3 + 64],
                                start=(t == 0 and j == 0),
                                stop=(t == 8 and j == CT - 1))
                    nc.vector.tensor_scalar_add(qk8[:, csl], pqk[:], sp[:, 0:1])
                    if c == 0:
                        for h in range(2):
                            nc.sync.dma_start(q_dr[:, h, 0:512],
                                              qk8[32 * h:32 * h + 32, 0:512])
                    if c % 2 == 1:
                        p0, p1 = 2 * (c - 1), 2 * (c - 1) + 4
                        hsl = slice(p0 * 256, p1 * 256)
                        for h in range(2):
                            if c == 3:
                                nc.sync.dma_start(
                                    q_dr[:, h, 512:2048],
                                    qk8[32 * h:32 * h + 32, 512:2048])
                            nc.sync.dma_start(
                                k_dr[:, p0:p1, :, h, :],
                                qk8[64 + 32 * h:96 + 32 * h, hsl].rearrange(
                                    "p (pr s j) -> p pr s j", s=2, j=128))
                    # v conv (fp8 DR, 9 passes per ct) -> bf16 -> xbar transpose
                    for ct in range(CT):
                        pv = psC.tile([128, 512], f32, tag="pc", name=f"pv{c}_{ct}")
                        for t in range(9):
                            nc.tensor.matmul(
                                pv[:], wv[:, t, ct, :, :],
                                xp8[:, :, base + t // 3:base + t // 3 + 8,
                                    t % 3:t % 3 + 64],
                                start=(t == 0), stop=(t == 8), perf_mode=DR)
                        nc.vector.tensor_scalar_mul(vsb[ct][:, csl], pv[:],
                                                    1.0 / WSCALE)
                        nc.sync.dma_start_transpose(
                            vt16[:, ct, c, :, :, :], vsb[ct][:, csl])
                        nc.gpsimd.tensor_copy(vt[:, ct, c, :, :, :],
                                              vt16[:, ct, c, :, :, :])
                    if c == 0:
                        emit_se(psT)
                    if c == 4:
                        nc.sync.dma_start(xq[:], xq_d[:])
                        nc.sync.dma_start(fusg[:], fusg_d[:])
                        nc.sync.dma_start(fusl[:], fusl_d[:])

                    if c >= 2:
                        for pl in range(2):
                            sc_stage(0, 2 * (c - 2) + pl, dacc[0], psT, pet)
                    if c >= 4:
                        for pl in range(2):
                            sc_stage(1, 2 * (c - 4) + pl, None, psT, pet, None)
                    if c == 7:
                        for pl in range(2):
                            sc_stage(3, pl, None, psT, pet, None)
                    if c >= 5:
                        for pl in range(2):
                            sc_stage(2, 2 * (c - 5) + pl, None, psT, pet, None)
                    if c >= 3:
                        for pl in range(2):
                            u_stage(0, 2 * (c - 3) + pl, pu0)

            with tc.tile_pool(name="psD", bufs=1, space="PSUM") as psD, \
                 tc.tile_pool(name="po2", bufs=3) as po2:
                po = po2
                # fold yse into glob fuse weights (Pool; after fusg DMA)
                for ct in range(CT):
                    nc.gpsimd.tensor_scalar_mul(fusg[:, 2 * ct:2 * ct + 2, :],
                                                fusg[:, 2 * ct:2 * ct + 2, :],
                                                yse[ct][:, 0:1])
                # flat pipeline over the remaining 56 sc / 58 u stages with
                # inline quarter finishes and per-quarter fuse
                all_sc = [(0, p) for p in range(12, 16)] + \
                         [(1, p) for p in range(8, 16)] + \
                         [(2, p) for p in range(6, 16)] + \
                         [(3, p) for p in range(2, NP)]
                all_u = [(0, p) for p in range(10, 16)] + \
                        [(1, p) for p in range(16)] + \
                        [(q, p) for q in range(2, NQ) for p in range(NP)]
                pus = {0: pu0}
                pDs = {}
                state = {"q": 0}
                for k in range(len(all_u)):
                    if k < len(all_sc):
                        q, p = all_sc[k]
                        sc_stage(q, p, None if q > 0 else dacc[0], psT, pet,
                                 None)
                    q, p = all_u[k]
                    if q > 0 and p == 0:
                        pus[q] = [psU.tile([128, 512], f32, tag=f"pu{t}",
                                           name=f"pu{t}_{q}")
                                  for t in range(CT)]
                        pDs[q] = psD.tile([32, 512], f32, tag="pD",
                                          name=f"pD{q}")
                    u_stage(q, p, pus[q], pDs.get(q))
                    if p == NP - 1:
                        finish_quarter(q, pus.pop(q), psD,
                                       dacc[0] if q == 0 else None,
                                       pDs.pop(q, None))
                        emit_fuse_quarter(q)

        with tc.tile_pool(name="podbg", bufs=1) as podbg:
            if debug:
                dbg_f = pw.tile([128, HW], f32, tag="dbgf")
                nc.vector.tensor_copy(dbg_f[:], qk8[:])
                nc.sync.dma_start(qk8_dbg[:], dbg_f[:])
                dbg_v = pw.tile([128, NP * 512], f32, tag="dbgv")
                nc.vector.tensor_copy(
                    dbg_v[:], vt[:].rearrange("p a b c d e -> p (a b c d e)"))
                nc.sync.dma_start(vt_dbg[:], dbg_v[:])
                dbg_d = pw.tile([1, HWh], f32, tag="dbgd")
                nc.vector.reciprocal(dbg_d[:], dr[:])
                nc.sync.dma_start(d_dbg[:], dbg_d[:])
                dbg_l = pw.tile([128, 2, HWh], f32, tag="dbgl")
                nc.vector.tensor_copy(dbg_l[:], loc16[:])
                nc.sync.dma_start(loc_dbg[:], dbg_l[:])

    nc.compile()
    return nc


def _prep_core_inputs(inputs):
    x = np.ascontiguousarray(inputs["x"], np.float32)
    wq = np.asarray(inputs["wq"], np.float32)
    bq = np.asarray(inputs["bq"], np.float32)
    wk = np.asarray(inputs["wk"], np.float32)
    bk = np.asarray(inputs["bk"], np.float32)
    wv_ = np.asarray(inputs["wv"], np.float32)
    bv = np.asarray(inputs["bv"], np.float32)
    fc1_w = np.asarray(inputs["fc1_w"], np.float32)
    fc1_b = np.asarray(inputs["fc1_b"], np.float32)
    fc2_w = np.asarray(inputs["fc2_w"], np.float32)
    fc2_b = np.asarray(inputs["fc2_b"], np.float32)
    fuse_w = np.asarray(inputs["fuse_w"], np.float32)[:, :, 0, 0]
    fuse_b = np.asarray(inputs["fuse_b"], np.float32)

    wqk = np.concatenate([wq, wk], axis=0)               # [128, 256, 3, 3]
    bqk = np.concatenate([bq, bk])[:, None].astype(np.float32)

    wqk16 = np.empty((128, 9, 2, 128), np.float32)
    wv8 = np.empty((128, 9, 2, 2, 128), np.float32)
    for t in range(9):
        dy, dx = t // 3, t % 3
        for j in range(CT):
            wqk16[:, t, j, :] = wqk[:, j * 128:(j + 1) * 128, dy, dx].T
            for cto in range(CT):
                wv8[:, t, cto, j, :] = (
                    wv_[cto * 128:(cto + 1) * 128, j * 128:(j + 1) * 128,
                        dy, dx].T * WSCALE)

    fusg = np.empty((128, 4, 128), np.float32)
    fusl = np.empty((128, 4, 128), np.float32)
    for ct in range(CT):
        for mt in range(CT):
            fusl[:, 2 * ct + mt, :] = fuse_w[mt * 128:(mt + 1) * 128,
                                             ct * 128:(ct + 1) * 128].T
            fusg[:, 2 * ct + mt, :] = fuse_w[mt * 128:(mt + 1) * 128,
                                             C + ct * 128:C + (ct + 1) * 128].T

    fuse_b_eff = fuse_b + fuse_w[:, :C] @ bv

    smallp = np.zeros((128, 39), np.float32)
    smallp[:, 0:1] = bqk
    smallp[:, 1:3] = np.stack([fuse_b_eff[t * 128:(t + 1) * 128]
                               for t in range(CT)], axis=1)
    smallp[:, 5:7] = np.stack([-fc2_b[t * 128:(t + 1) * 128]
                               for t in range(CT)], axis=1)
    for j in range(CT):
        smallp[:, 7 + j * 16:7 + (j + 1) * 16] = (fc1_w / HW)[:, j * 128:(j + 1) * 128].T
    smallq = np.zeros((16, 257), np.float32)
    for t in range(CT):
        smallq[:, t * 128:(t + 1) * 128] = fc2_w[t * 128:(t + 1) * 128, :].T
    smallq[:, 256] = fc1_b

    shared = dict(
        wqk=np.ascontiguousarray(wqk16).astype(b16np),
        wv=np.ascontiguousarray(wv8).astype(f8np),
        fusg=np.ascontiguousarray(fusg).astype(b16np),
        fusl=np.ascontiguousarray(fusl).astype(b16np),
        smallp=smallp, smallq=smallq,
    )

    in_maps = []
    for core in range(N_CORES):
        s, p = divmod(core, 2)
        s0 = p * 32
        t0 = (s0 + 32) % 64
        P = np.zeros((C, 66, 66), np.float32)
        P[:, 1:65, 1:65] = x[s]
        xp = np.concatenate([P[:, s0:s0 + 34], P[:, t0:t0 + 34]], axis=1)
        xp4 = xp.reshape(2, 128, 68, 66).transpose(1, 0, 2, 3)
        xqc = xp[:, 1:33, 1:65].reshape(2, 128, 32, 64).transpose(1, 0, 2, 3)
        m = dict(shared)
        m["xp16"] = np.ascontiguousarray(xp4).astype(b16np)
        m["xp8"] = np.ascontiguousarray(xp4).astype(f8np)
        m["xq"] = np.ascontiguousarray(xqc).astype(b16np)
        in_maps.append(m)
    return in_maps


def kernel(**inputs):
    global _compiled
    if _compiled is None:
        _compiled = _build()
    nc = _compiled
    in_maps = _prep_core_inputs(inputs)
    res = run_bass_kernel_spmd(nc, in_maps, list(range(N_CORES)))
    out = np.empty((4, C, H, W), np.float32)
    for core in range(N_CORES):
        s, p = divmod(core, 2)
        o = res.results[core]["out"]          # [2, 128, 2048]
        out[s, :, p * 32:(p + 1) * 32, :] = o.reshape(C, 32, 64)
    return out


# revision 34
# speedup vs baseline: 1.0950x; 1.0950x over previous
"""DualAttention Trainium2 kernel (fp8 DoubleRow + bf16 qk conv).

Sharding: 8 cores = 4 samples x 2 query-halves. Per core the sample image is
"rolled" by the half offset (host-side, with correct zero padding), so every
core runs the identical program on its first 2048 query positions; attention
over key positions is permutation-invariant, so convs/attention on the rolled
image give the true result for the core's half.

Precision (tolerance is 2e-2 max-abs / absmax):
  qk conv   bf16 (fp8 conv noise on scores blows the error budget)
  v conv    fp8e4 DoubleRow, weights x16 (fp8 min-normal), /16 on convert
  scores    fp8e4 DoubleRow over channel halves (K=32), q/k requantized fp8
  exp       ACT, scale 1/8, bias -2 (keeps E' = e^(s-2) inside fp8's 240)
  U = vT E  fp8e4 DoubleRow over j-tile pairs
  D         quarter 0: Pool dacc + PE colsum; quarters 1-3: ones-matmul
  fuse      bf16 (glob path feeds x straight to the output)
The e^-2 shift cancels between U and D. The SE yse gate is folded into the
glob fuse weights on-chip, so glob = x * yse is never materialized. v is
transposed by chunked xbar DMA transposes (bf16), then Pool converts to fp8.

Schedule: conv chunks and attention are software-pipelined. qk8->q_dr/k_dr
SBUF re-layout DMAs fire every other chunk so scores for quarters 0-3 start
as early as their k-tiles exist; exp runs on an otherwise-clean ACT queue
(the 64 exp instructions are the ~67us critical resource). U lags scores by
several stages; the remaining pairs run as one flat cross-quarter pipeline
with quarter finishes (colsum/ones-matmul -> reciprocal -> ones-broadcast ->
loc16) and the per-quarter fuse emitted inline.
"""

import sys

sys.path.insert(0, "/opt/trn_rl_repo")

import numpy as np
import ml_dtypes

import concourse.bass as bass
import concourse.mybir as mybir
import concourse.tile as tile
from concourse import bacc
from concourse.bass_utils import run_bass_kernel_spmd

f32 = mybir.dt.float32
f32r = mybir.dt.float32r
bf16 = mybir.dt.bfloat16
fp8 = mybir.dt.float8e4
AF = mybir.ActivationFunctionType
DR = mybir.MatmulPerfMode.DoubleRow
f8np = ml_dtypes.float8_e4m3
b16np = ml_dtypes.bfloat16

C = 256
CT = 2          # channel tiles of 128
H = W = 64
HW = H * W      # 4096
HWh = 2048      # query positions per core
NP = 16         # j-tile pairs (256 key positions each)
NQ = 4          # i quarters of 512 query positions
N_CORES = 8
WSCALE = 16.0   # host-side v-conv weight scale (fp8 subnormal avoidance)
ESHIFT = -2.0   # exp bias: E' = exp(s + ESHIFT)

_compiled = None

# (chunk, base_row) for the 8 x 512-position conv chunks; the rolled image
# has two 34-row blocks (rows 0:34 own half, 34:68 other half).
CHUNKS = [(c, c * 8 if c < 4 else 34 + (c - 4) * 8) for c in range(8)]


def _build(debug=False):
    nc = bacc.Bacc("TRN2", target_bir_lowering=False, debug=False,
                   num_devices=N_CORES)

    xp16_d = nc.declare_dram_parameter("xp16", [128, 2, 68, 66], bf16, isOutput=False)
    xp8_d = nc.declare_dram_parameter("xp8", [128, 2, 68, 66], fp8, isOutput=False)
    xq_d = nc.declare_dram_parameter("xq", [128, 2, 32, 64], bf16, isOutput=False)
    wqk_d = nc.declare_dram_parameter("wqk", [128, 9, 2, 128], bf16, isOutput=False)
    wv_d = nc.declare_dram_parameter("wv", [128, 9, 2, 2, 128], fp8, isOutput=False)
    fusg_d = nc.declare_dram_parameter("fusg", [128, 4, 128], bf16, isOutput=False)
    fusl_d = nc.declare_dram_parameter("fusl", [128, 4, 128], bf16, isOutput=False)
    smallp_d = nc.declare_dram_parameter("smallp", [128, 39], f32, isOutput=False)
    smallq_d = nc.declare_dram_parameter("smallq", [16, 257], f32, isOutput=False)
    out_d = nc.declare_dram_parameter("out", [2, 128, HWh], f32, isOutput=True)
    if debug:
        qk8_dbg = nc.declare_dram_parameter("qk8_dbg", [128, HW], f32, isOutput=True)
        vt_dbg = nc.declare_dram_parameter("vt_dbg", [128, NP * 512], f32, isOutput=True)
        d_dbg = nc.declare_dram_parameter("d_dbg", [1, HWh], f32, isOutput=True)
        loc_dbg = nc.declare_dram_parameter("loc_dbg", [128, 2, HWh], f32, isOutput=True)

    with tile.TileContext(nc) as tc, \
         nc.allow_low_precision(reason="fp8/bf16 storage; tolerance 2e-2"):
      with tc.tile_pool(name="pw", bufs=1) as pw:
        # ---- persistent tiles
        wqk = pw.tile([128, 9, 2, 128], bf16)
        wv = pw.tile([128, 9, 2, 2, 128], fp8)
        fusg = pw.tile([128, 4, 128], bf16)
        fusl = pw.tile([128, 4, 128], bf16)
        sp = pw.tile([128, 39], f32)
        sq = pw.tile([16, 257], f32)
        xp16 = pw.tile([128, 2, 68, 66], bf16)
        xp8 = pw.tile([128, 2, 68, 66], fp8)
        xq = pw.tile([128, 2, 32, 64], bf16)
        qk8 = pw.tile([128, HW], fp8)
        q_dr = pw.tile([32, 2, HWh], fp8)
        k_dr = pw.tile([32, NP, 2, 2, 128], fp8)        # (pair, sub, chanhalf, j)
        vt16 = pw.tile([128, 2, 8, 2, 2, 128], bf16)    # (ct, chunk, pl, s, c)
        vt = pw.tile([128, 2, 8, 2, 2, 128], fp8)
        vsb = [pw.tile([128, HW], bf16, tag=f"vsb{t}", name=f"vsb{t}")
               for t in range(CT)]
        loc16 = pw.tile([128, 2, HWh], bf16)            # (ct, i)
        ones8 = pw.tile([128, 2, 32], fp8)
        oner = pw.tile([1, 128], f32r)
        onerf = pw.tile([1, 128], f32)
        onecf = pw.tile([128, 1], f32)
        onec = pw.tile([128, 1], f32r)
        ebias = pw.tile([128, 1], f32)
        dr = pw.tile([1, HWh], f32r)
        dacc = [pw.tile([128, 512], f32r, tag=f"dacc{i}", name=f"dacc{i}")
                for i in range(4)]
        recb = [pw.tile([128, 512], f32, tag=f"recb{i}", name=f"recb{i}")
                for i in range(2)]
        yse = [pw.tile([128, 1], f32, tag=f"yse{t}", name=f"yse{t}")
               for t in range(CT)]
        sums = [pw.tile([128, 1], f32, tag=f"sums{t}", name=f"sums{t}")
                for t in range(CT)]

        # ---- input DMAs (first conv chunk needs wqk + xp16 rows 0:10)
        nc.gpsimd.dma_start(wqk[:, 0:1, :, :], wqk_d[:, 0:1, :, :])
        nc.sync.dma_start(xp16[:, :, 0:6, :], xp16_d[:, :, 0:6, :])
        nc.gpsimd.dma_start(wqk[:, 1:3, :, :], wqk_d[:, 1:3, :, :])
        nc.sync.dma_start(xp16[:, :, 6:12, :], xp16_d[:, :, 6:12, :])
        nc.sync.dma_start(wqk[:, 3:9, :, :], wqk_d[:, 3:9, :, :])
        nc.sync.dma_start(xp8[:, :, 0:12, :], xp8_d[:, :, 0:12, :])
        nc.sync.dma_start(wv[:], wv_d[:])
        nc.sync.dma_start(sp[:], smallp_d[:])
        nc.sync.dma_start(sq[:], smallq_d[:])
        for r0, r1 in [(12, 23), (23, 34), (34, 46), (46, 57), (57, 68)]:
            nc.sync.dma_start(xp16[:, :, r0:r1, :], xp16_d[:, :, r0:r1, :])
            nc.sync.dma_start(xp8[:, :, r0:r1, :], xp8_d[:, :, r0:r1, :])

        nc.vector.memset(ones8[:], 1.0)
        nc.vector.memset(onerf[:], 1.0)
        nc.vector.tensor_copy(oner[:], onerf[:])
        nc.vector.memset(onecf[:], 1.0)
        nc.vector.tensor_copy(onec[:], onecf[:])
        nc.vector.memset(ebias[:], ESHIFT)

        ets = {}

        def sc_stage(q, pr, dacc_t, psT, pet, pD=None):
            """scores -> exp (fp8 et); D via DVE dacc (q0) or pD matmul."""
            isl = slice(q * 512, (q + 1) * 512)
            pT = psT.tile([128, 2, 512], f32, tag="pT", name=f"pT{q}_{pr}")
            for s in range(2):
                nc.tensor.matmul(pT[:, s, :], k_dr[:, pr, s, :, :],
                                 q_dr[:, :, isl], start=True, stop=True,
                                 perf_mode=DR)
            et = pet.tile([128, 2, 512], fp8, tag="et", name=f"et{q}_{pr}")
            nc.scalar.activation(et[:], pT[:], AF.Exp, bias=ebias[:, 0:1],
                                 scale=0.125)
            if dacc_t is not None:
                eng = nc.gpsimd
                if pr == 0:
                    eng.tensor_copy(dacc_t[:], et[:, 0, :])
                else:
                    eng.tensor_add(dacc_t[:], dacc_t[:], et[:, 0, :])
                eng.tensor_add(dacc_t[:], dacc_t[:], et[:, 1, :])
            ets[(q, pr)] = (et, pD)

        def u_stage(q, pr, pu, pD=None):
            et, _ = ets.pop((q, pr))
            for t in range(CT):
                nc.tensor.matmul(pu[t][:], vt[:, t, pr // 2, pr % 2, :, :],
                                 et[:], start=(pr == 0), stop=(pr == NP - 1),
                                 perf_mode=DR)
            if pD is not None:
                nc.tensor.matmul(pD[:], ones8[:], et[:],
                                 start=(pr == 0), stop=(pr == NP - 1),
                                 perf_mode=DR)

        def finish_quarter(q, pu, psD, dacc_t, pD=None):
            isl = slice(q * 512, (q + 1) * 512)
            if dacc_t is not None:
                pD = psD.tile([32, 512], f32, tag="pB", name=f"pDc{q}")
                nc.tensor.matmul(pD[0:1, :], onec[:], dacc_t[:],
                                 start=True, stop=True)
            nc.vector.reciprocal(dr[:, isl], pD[0:1, :])
            pB = psD.tile([128, 512], f32, tag="pB", name=f"pB{q}")
            nc.tensor.matmul(pB[:], oner[:], dr[:, isl], start=True, stop=True)
            rb = recb[q % 2]
            nc.vector.tensor_copy(rb[:], pB[:])
            for t in range(CT):
                nc.vector.tensor_mul(loc16[:, t, isl], pu[t][:], rb[:])

        po = None

        def emit_fuse_quarter(ic):
            isl = slice(ic * 512, (ic + 1) * 512)
            for mt in range(CT):
                pf = psD.tile([128, 512], f32, tag="pB", name=f"pf{ic}_{mt}")
                nc.tensor.matmul(pf[:], fusl[:, 0 + mt, :], loc16[:, 0, isl],
                                 start=True, stop=False)
                nc.tensor.matmul(pf[:], fusl[:, 2 + mt, :], loc16[:, 1, isl],
                                 start=False, stop=False)
                nc.tensor.matmul(pf[:], fusg[:, 0 + mt, :],
                                 xq[:, 0, ic * 8:(ic + 1) * 8, :],
                                 start=False, stop=False)
                nc.tensor.matmul(pf[:], fusg[:, 2 + mt, :],
                                 xq[:, 1, ic * 8:(ic + 1) * 8, :],
                                 start=False, stop=True)
                ob = po.tile([128, 512], f32, tag="ob", name=f"ob{ic}_{mt}")
                nc.vector.tensor_scalar_add(ob[:], pf[:], sp[:, 1 + mt:2 + mt])
                nc.sync.dma_start(out_d[mt, :, isl], ob[:])

        with tc.tile_pool(name="pet", bufs=24) as pet, \
             tc.tile_pool(name="psT", bufs=2, space="PSUM") as psT, \
             tc.tile_pool(name="psU", bufs=1, space="PSUM") as psU:
            pu0 = [psU.tile([128, 512], f32, tag=f"pu{t}", name=f"pu{t}_0")
                   for t in range(CT)]
            def emit_se(psSE):
                def se_psum(name):
                    t = psSE.tile([128, 2, 512], f32, tag="pT", name=name)
                    return t
                # ---- SE channel sums on ACT (accum_out); scratch fp8 output
                sa = pw.tile([128, 1], f32)
                sb_ = pw.tile([128, 1], f32)
                scr = pw.tile([128, 32, 64], fp8, tag="sescr")
                for j in range(CT):
                    nc.scalar.activation(scr[:], xp8[:, j, 1:33, 1:65],
                                         AF.Copy, accum_out=sa[:])
                    nc.scalar.activation(scr[:], xp8[:, j, 35:67, 1:65],
                                         AF.Copy, accum_out=sb_[:])
                    nc.vector.tensor_add(sums[j][:], sa[:], sb_[:])
                # ---- SE MLP: yse = sigmoid(fc2 @ relu(fc1 @ mean + b1) + b2)
                ps1 = se_psum("ps1")
                for j in range(CT):
                    nc.tensor.matmul(ps1[0:16, 0, 0:1],
                                     sp[:, 7 + j * 16:7 + (j + 1) * 16],
                                     sums[j][:], start=(j == 0),
                                     stop=(j == CT - 1))
                y1 = pw.tile([16, 1], f32)
                nc.scalar.activation(y1[:], ps1[0:16, 0, 0:1], AF.Relu,
                                     bias=sq[0:16, 256:257])
                for t in range(CT):
                    ps2 = se_psum(f"ps2_{t}")
                    nc.tensor.matmul(ps2[:, 0, 0:1],
                                     sq[0:16, t * 128:(t + 1) * 128],
                                     y1[:], start=True, stop=True)
                    # sigmoid(z) = 1/(1+exp(-z)); sp col 5+t holds -fc2_b
                    en = pw.tile([128, 1], f32, tag="en")
                    nc.scalar.activation(en[:], ps2[:, 0, 0:1], AF.Exp,
                                         bias=sp[:, 5 + t:6 + t], scale=-1.0)
                    nc.vector.tensor_scalar_add(en[:], en[:], 1.0)
                    nc.vector.reciprocal(yse[t][:], en[:])
            with tc.tile_pool(name="psC", bufs=2, space="PSUM") as psC:
                # ---- conv chunks; quarter-0 attention software-pipelined in
                # (scores lag 2 chunks, U lags 3, so PE never waits on the
                # qk8->k_dr DMA chain or on ACT's exp)
                for c, base in CHUNKS:
                    csl = slice(c * 512, (c + 1) * 512)
                    # qk conv (bf16, 18 passes)
                    pqk = psC.tile([128, 512], f32, tag="pc", name=f"pqk{c}")
                    for t in range(9):
                        for j in range(CT):
                            nc.tensor.matmul(
                                pqk[:], wqk[:, t, j, :],
                                xp16[:, j, base + t // 3:base + t // 3 + 8,
                                     t % 3:t % 3 + 64],
                                start=(t == 0 and j == 0),
                                stop=(t == 8 and j == CT - 1))
                    nc.vector.tensor_scalar_add(qk8[:, csl], pqk[:], sp[:, 0:1])
                    if c == 0:
                        for h in range(2):
                            nc.sync.dma_start(q_dr[:, h, 0:512],
                                              qk8[32 * h:32 * h + 32, 0:512])
                    if c % 2 == 1:
                        p0, p1 = 2 * (c - 1), 2 * (c - 1) + 4
                        hsl = slice(p0 * 256, p1 * 256)
                        for h in range(2):
                            if c == 3:
                                nc.sync.dma_start(
                                    q_dr[:, h, 512:2048],
                                    qk8[32 * h:32 * h + 32, 512:2048])
                            nc.sync.dma_start(
                                k_dr[:, p0:p1, :, h, :],
                                qk8[64 + 32 * h:96 + 32 * h, hsl].rearrange(
                                    "p (pr s j) -> p pr s j", s=2, j=128))
                    # v conv (fp8 DR, 9 passes per ct) -> bf16 -> xbar transpose
                    for ct in range(CT):
                        pv = psC.tile([128, 512], f32, tag="pc", name=f"pv{c}_{ct}")
                        for t in range(9):
                            nc.tensor.matmul(
                                pv[:], wv[:, t, ct, :, :],
                                xp8[:, :, base + t // 3:base + t // 3 + 8,
                                    t % 3:t % 3 + 64],
                                start=(t == 0), stop=(t == 8), perf_mode=DR)
                        nc.vector.tensor_scalar_mul(vsb[ct][:, csl], pv[:],
                                                    1.0 / WSCALE)
                        nc.sync.dma_start_transpose(
                            vt16[:, ct, c, :, :, :], vsb[ct][:, csl])
                        nc.gpsimd.tensor_copy(vt[:, ct, c, :, :, :],
                                              vt16[:, ct, c, :, :, :])
                    if c == 0:
                        emit_se(psT)
                    if c == 4:
                        nc.sync.dma_start(xq[:], xq_d[:])
                        nc.sync.dma_start(fusg[:], fusg_d[:])
                        nc.sync.dma_start(fusl[:], fusl_d[:])

                    if c >= 2:
                        for pl in range(2):
                            sc_stage(0, 2 * (c - 2) + pl, dacc[0], psT, pet)
                    if c >= 4:
                        for pl in range(2):
                            sc_stage(1, 2 * (c - 4) + pl, None, psT, pet, None)
                    if c == 7:
                        for pl in range(2):
                            sc_stage(3, pl, None, psT, pet, None)
                    if c >= 5:
                        for pl in range(2):
                            sc_stage(2, 2 * (c - 5) + pl, None, psT, pet, None)
                    if c >= 3:
                        for pl in range(2):
                            u_stage(0, 2 * (c - 3) + pl, pu0)

            with tc.tile_pool(name="psD", bufs=1, space="PSUM") as psD, \
                 tc.tile_pool(name="po2", bufs=3) as po2:
                po = po2
                # fold yse into glob fuse weights (Pool; after fusg DMA)
                for ct in range(CT):
                    nc.gpsimd.tensor_scalar_mul(fusg[:, 2 * ct:2 * ct + 2, :],
                                                fusg[:, 2 * ct:2 * ct + 2, :],
                                                yse[ct][:, 0:1])
                # flat pipeline over the remaining 56 sc / 58 u stages with
                # inline quarter finishes and per-quarter fuse
                all_sc = [(0, p) for p in range(12, 16)] + \
                         [(1, p) for p in range(8, 16)] + \
                         [(2, p) for p in range(6, 16)] + \
                         [(3, p) for p in range(2, NP)]
                all_u = [(0, p) for p in range(10, 16)] + \
                        [(1, p) for p in range(16)] + \
                        [(q, p) for q in range(2, NQ) for p in range(NP)]
                pus = {0: pu0}
                pDs = {}
                state = {"q": 0}
                pending = []
                for k in range(len(all_u) + 4):
                    if k < len(all_sc):
                        q, p = all_sc[k]
                        sc_stage(q, p, None if q > 0 else dacc[0], psT, pet,
                                 None)
                    if k < len(all_u):
                        q, p = all_u[k]
                        if q > 0 and p == 0:
                            pus[q] = [psU.tile([128, 512], f32, tag=f"pu{t}",
                                               name=f"pu{t}_{q}")
                                      for t in range(CT)]
                            pDs[q] = psD.tile([32, 512], f32, tag="pD",
                                              name=f"pD{q}")
                        u_stage(q, p, pus[q], pDs.get(q))
                        if p == NP - 1:
                            pending.append((k + 4, q))
                    while pending and pending[0][0] <= k:
                        _, fq = pending.pop(0)
                        finish_quarter(fq, pus.pop(fq), psD,
                                       dacc[0] if fq == 0 else None,
                                       pDs.pop(fq, None))
                        emit_fuse_quarter(fq)

        with tc.tile_pool(name="podbg", bufs=1) as podbg:
            if debug:
                dbg_f = pw.tile([128, HW], f32, tag="dbgf")
                nc.vector.tensor_copy(dbg_f[:], qk8[:])
                nc.sync.dma_start(qk8_dbg[:], dbg_f[:])
                dbg_v = pw.tile([128, NP * 512], f32, tag="dbgv")
                nc.vector.tensor_copy(
                    dbg_v[:], vt[:].rearrange("p a b c d e -> p (a b c d e)"))
                nc.sync.dma_start(vt_dbg[:], dbg_v[:])
                dbg_d = pw.tile([1, HWh], f32, tag="dbgd")
                nc.vector.reciprocal(dbg_d[:], dr[:])
                nc.sync.dma_start(d_dbg[:], dbg_d[:])
                dbg_l = pw.tile([128, 2, HWh], f32, tag="dbgl")
                nc.vector.tensor_copy(dbg_l[:], loc16[:])
                nc.sync.dma_start(loc_dbg[:], dbg_l[:])

    nc.compile()
    return nc


def _prep_core_inputs(inputs):
    x = np.ascontiguousarray(inputs["x"], np.float32)
    wq = np.asarray(inputs["wq"], np.float32)
    bq = np.asarray(inputs["bq"], np.float32)
    wk = np.asarray(inputs["wk"], np.float32)
    bk = np.asarray(inputs["bk"], np.float32)
    wv_ = np.asarray(inputs["wv"], np.float32)
    bv = np.asarray(inputs["bv"], np.float32)
    fc1_w = np.asarray(inputs["fc1_w"], np.float32)
    fc1_b = np.asarray(inputs["fc1_b"], np.float32)
    fc2_w = np.asarray(inputs["fc2_w"], np.float32)
    fc2_b = np.asarray(inputs["fc2_b"], np.float32)
    fuse_w = np.asarray(inputs["fuse_w"], np.float32)[:, :, 0, 0]
    fuse_b = np.asarray(inputs["fuse_b"], np.float32)

    wqk = np.concatenate([wq, wk], axis=0)               # [128, 256, 3, 3]
    bqk = np.concatenate([bq, bk])[:, None].astype(np.float32)

    wqk16 = np.empty((128, 9, 2, 128), np.float32)
    wv8 = np.empty((128, 9, 2, 2, 128), np.float32)
    for t in range(9):
        dy, dx = t // 3, t % 3
        for j in range(CT):
            wqk16[:, t, j, :] = wqk[:, j * 128:(j + 1) * 128, dy, dx].T
            for cto in range(CT):
                wv8[:, t, cto, j, :] = (
                    wv_[cto * 128:(cto + 1) * 128, j * 128:(j + 1) * 128,
                        dy, dx].T * WSCALE)

    fusg = np.empty((128, 4, 128), np.float32)
    fusl = np.empty((128, 4, 128), np.float32)
    for ct in range(CT):
        for mt in range(CT):
            fusl[:, 2 * ct + mt, :] = fuse_w[mt * 128:(mt + 1) * 128,
                                             ct * 128:(ct + 1) * 128].T
            fusg[:, 2 * ct + mt, :] = fuse_w[mt * 128:(mt + 1) * 128,
                                             C + ct * 128:C + (ct + 1) * 128].T

    fuse_b_eff = fuse_b + fuse_w[:, :C] @ bv

    smallp = np.zeros((128, 39), np.float32)
    smallp[:, 0:1] = bqk
    smallp[:, 1:3] = np.stack([fuse_b_eff[t * 128:(t + 1) * 128]
                               for t in range(CT)], axis=1)
    smallp[:, 5:7] = np.stack([-fc2_b[t * 128:(t + 1) * 128]
                               for t in range(CT)], axis=1)
    for j in range(CT):
        smallp[:, 7 + j * 16:7 + (j + 1) * 16] = (fc1_w / HW)[:, j * 128:(j + 1) * 128].T
    smallq = np.zeros((16, 257), np.float32)
    for t in range(CT):
        smallq[:, t * 128:(t + 1) * 128] = fc2_w[t * 128:(t + 1) * 128, :].T
    smallq[:, 256] = fc1_b

    shared = dict(
        wqk=np.ascontiguousarray(wqk16).astype(b16np),
        wv=np.ascontiguousarray(wv8).astype(f8np),
        fusg=np.ascontiguousarray(fusg).astype(b16np),
        fusl=np.ascontiguousarray(fusl).astype(b16np),
        smallp=smallp, smallq=smallq,
    )

    in_maps = []
    for core in range(N_CORES):
        s, p = divmod(core, 2)
        s0 = p * 32
        t0 = (s0 + 32) % 64
        P = np.zeros((C, 66, 66), np.float32)
        P[:, 1:65, 1:65] = x[s]
        xp = np.concatenate([P[:, s0:s0 + 34], P[:, t0:t0 + 34]], axis=1)
        xp4 = xp.reshape(2, 128, 68, 66).transpose(1, 0, 2, 3)
        xqc = xp[:, 1:33, 1:65].reshape(2, 128, 32, 64).transpose(1, 0, 2, 3)
        m = dict(shared)
        m["xp16"] = np.ascontiguousarray(xp4).astype(b16np)
        m["xp8"] = np.ascontiguousarray(xp4).astype(f8np)
        m["xq"] = np.ascontiguousarray(xqc).astype(b16np)
        in_maps.append(m)
    return in_maps


def kernel(**inputs):
    global _compiled
    if _compiled is None:
        _compiled = _build()
    nc = _compiled
    in_maps = _prep_core_inputs(inputs)
    res = run_bass_kernel_spmd(nc, in_maps, list(range(N_CORES)))
    out = np.empty((4, C, H, W), np.float32)
    for core in range(N_CORES):
        s, p = divmod(core, 2)
        o = res.results[core]["out"]          # [2, 128, 2048]
        out[s, :, p * 32:(p + 1) * 32, :] = o.reshape(C, 32, 64)
    return out


# revision 35
# speedup vs baseline: 1.0992x; 1.0038x over previous
"""DualAttention Trainium2 kernel (fp8 DoubleRow + bf16 qk conv).

Sharding: 8 cores = 4 samples x 2 query-halves. Per core the sample image is
"rolled" by the half offset (host-side, with correct zero padding), so every
core runs the identical program on its first 2048 query positions; attention
over key positions is permutation-invariant, so convs/attention on the rolled
image give the true result for the core's half.

Precision (tolerance is 2e-2 max-abs / absmax):
  qk conv   bf16 (fp8 conv noise on scores blows the error budget)
  v conv    fp8e4 DoubleRow, weights x16 (fp8 min-normal), /16 on convert
  scores    fp8e4 DoubleRow over channel halves (K=32), q/k requantized fp8
  exp       ACT, scale 1/8, bias -2 (keeps E' = e^(s-2) inside fp8's 240)
  U = vT E  fp8e4 DoubleRow over j-tile pairs
  D         quarter 0: Pool dacc + PE colsum; quarters 1-3: ones-matmul
  fuse      bf16 (glob path feeds x straight to the output)
The e^-2 shift cancels between U and D. The SE yse gate is folded into the
glob fuse weights on-chip, so glob = x * yse is never materialized. v is
transposed by chunked xbar DMA transposes (bf16), then Pool converts to fp8.

Schedule: conv chunks and attention are software-pipelined. qk8->q_dr/k_dr
SBUF re-layout DMAs fire every other chunk so scores for quarters 0-3 start
as early as their k-tiles exist; exp runs on an otherwise-clean ACT queue
(the 64 exp instructions are the ~67us critical resource). U lags scores by
several stages; the remaining pairs run as one flat cross-quarter pipeline
with quarter finishes (colsum/ones-matmul -> reciprocal -> ones-broadcast ->
loc16) and the per-quarter fuse emitted inline.
"""

import sys

sys.path.insert(0, "/opt/trn_rl_repo")

import numpy as np
import ml_dtypes

import concourse.bass as bass
import concourse.mybir as mybir
import concourse.tile as tile
from concourse import bacc
from concourse.bass_utils import run_bass_kernel_spmd

f32 = mybir.dt.float32
f32r = mybir.dt.float32r
bf16 = mybir.dt.bfloat16
fp8 = mybir.dt.float8e4
AF = mybir.ActivationFunctionType
DR = mybir.MatmulPerfMode.DoubleRow
f8np = ml_dtypes.float8_e4m3
b16np = ml_dtypes.bfloat16

C = 256
CT = 2          # channel tiles of 128
H = W = 64
HW = H * W      # 4096
HWh = 2048      # query positions per core
NP = 16         # j-tile pairs (256 key positions each)
NQ = 4          # i quarters of 512 query positions
N_CORES = 8
WSCALE = 16.0   # host-side v-conv weight scale (fp8 subnormal avoidance)
ESHIFT = -2.0   # exp bias: E' = exp(s + ESHIFT)

_compiled = None

# (chunk, base_row) for the 8 x 512-position conv chunks; the rolled image
# has two 34-row blocks (rows 0:34 own half, 34:68 other half).
CHUNKS = [(c, c * 8 if c < 4 else 34 + (c - 4) * 8) for c in range(8)]


def _build(debug=False):
    nc = bacc.Bacc("TRN2", target_bir_lowering=False, debug=False,
                   num_devices=N_CORES)

    xp16_d = nc.declare_dram_parameter("xp16", [128, 2, 68, 66], bf16, isOutput=False)
    xp8_d = nc.declare_dram_parameter("xp8", [128, 2, 68, 66], fp8, isOutput=False)
    xq_d = nc.declare_dram_parameter("xq", [128, 2, 32, 64], bf16, isOutput=False)
    wqk_d = nc.declare_dram_parameter("wqk", [128, 9, 2, 128], bf16, isOutput=False)
    wv_d = nc.declare_dram_parameter("wv", [128, 9, 2, 2, 128], fp8, isOutput=False)
    fusg_d = nc.declare_dram_parameter("fusg", [128, 4, 128], bf16, isOutput=False)
    fusl_d = nc.declare_dram_parameter("fusl", [128, 4, 128], bf16, isOutput=False)
    smallp_d = nc.declare_dram_parameter("smallp", [128, 39], f32, isOutput=False)
    smallq_d = nc.declare_dram_parameter("smallq", [16, 257], f32, isOutput=False)
    out_d = nc.declare_dram_parameter("out", [2, 128, HWh], f32, isOutput=True)
    if debug:
        qk8_dbg = nc.declare_dram_parameter("qk8_dbg", [128, HW], f32, isOutput=True)
        vt_dbg = nc.declare_dram_parameter("vt_dbg", [128, NP * 512], f32, isOutput=True)
        d_dbg = nc.declare_dram_parameter("d_dbg", [1, HWh], f32, isOutput=True)
        loc_dbg = nc.declare_dram_parameter("loc_dbg", [128, 2, HWh], f32, isOutput=True)

    with tile.TileContext(nc) as tc, \
         nc.allow_low_precision(reason="fp8/bf16 storage; tolerance 2e-2"):
      with tc.tile_pool(name="pw", bufs=1) as pw:
        # ---- persistent tiles
        wqk = pw.tile([128, 9, 2, 128], bf16)
        wv = pw.tile([128, 9, 2, 2, 128], fp8)
        fusg = pw.tile([128, 4, 128], bf16)
        fusl = pw.tile([128, 4, 128], bf16)
        sp = pw.tile([128, 39], f32)
        sq = pw.tile([16, 257], f32)
        xp16 = pw.tile([128, 2, 68, 66], bf16)
        xp8 = pw.tile([128, 2, 68, 66], fp8)
        xq = pw.tile([128, 2, 32, 64], bf16)
        qk8 = pw.tile([128, HW], fp8)
        q_dr = pw.tile([32, 2, HWh], fp8)
        k_dr = pw.tile([32, NP, 2, 2, 128], fp8)        # (pair, sub, chanhalf, j)
        vt16 = pw.tile([128, 2, 8, 2, 2, 128], bf16)    # (ct, chunk, pl, s, c)
        vt = pw.tile([128, 2, 8, 2, 2, 128], fp8)
        vsb = [pw.tile([128, HW], bf16, tag=f"vsb{t}", name=f"vsb{t}")
               for t in range(CT)]
        loc16 = pw.tile([128, 2, HWh], bf16)            # (ct, i)
        ones8 = pw.tile([128, 2, 32], fp8)
        oner = pw.tile([1, 128], f32r)
        onerf = pw.tile([1, 128], f32)
        onecf = pw.tile([128, 1], f32)
        onec = pw.tile([128, 1], f32r)
        ebias = pw.tile([128, 1], f32)
        dr = pw.tile([1, HWh], f32r)
        dacc = [pw.tile([128, 512], f32r, tag=f"dacc{i}", name=f"dacc{i}")
                for i in range(4)]
        recb = [pw.tile([128, 512], f32, tag=f"recb{i}", name=f"recb{i}")
                for i in range(2)]
        yse = [pw.tile([128, 1], f32, tag=f"yse{t}", name=f"yse{t}")
               for t in range(CT)]
        sums = [pw.tile([128, 1], f32, tag=f"sums{t}", name=f"sums{t}")
                for t in range(CT)]

        # ---- input DMAs (first conv chunk needs wqk + xp16 rows 0:10)
        nc.gpsimd.dma_start(wqk[:, 0:1, :, :], wqk_d[:, 0:1, :, :])
        nc.sync.dma_start(xp16[:, :, 0:6, :], xp16_d[:, :, 0:6, :])
        nc.gpsimd.dma_start(wqk[:, 1:3, :, :], wqk_d[:, 1:3, :, :])
        nc.sync.dma_start(xp16[:, :, 6:12, :], xp16_d[:, :, 6:12, :])
        nc.sync.dma_start(wqk[:, 3:9, :, :], wqk_d[:, 3:9, :, :])
        nc.sync.dma_start(xp8[:, :, 0:12, :], xp8_d[:, :, 0:12, :])
        nc.sync.dma_start(wv[:], wv_d[:])
        nc.sync.dma_start(sp[:], smallp_d[:])
        nc.sync.dma_start(sq[:], smallq_d[:])
        for r0, r1 in [(12, 23), (23, 34), (34, 46), (46, 57), (57, 68)]:
            nc.sync.dma_start(xp16[:, :, r0:r1, :], xp16_d[:, :, r0:r1, :])
            nc.sync.dma_start(xp8[:, :, r0:r1, :], xp8_d[:, :, r0:r1, :])

        nc.vector.memset(ones8[:], 1.0)
        nc.vector.memset(onerf[:], 1.0)
        nc.vector.tensor_copy(oner[:], onerf[:])
        nc.vector.memset(onecf[:], 1.0)
        nc.vector.tensor_copy(onec[:], onecf[:])
        nc.vector.memset(ebias[:], ESHIFT)

        ets = {}

        def sc_stage(q, pr, dacc_t, psT, pet, pD=None):
            """scores -> exp (fp8 et); D via DVE dacc (q0) or pD matmul."""
            isl = slice(q * 512, (q + 1) * 512)
            pT = psT.tile([128, 2, 512], f32, tag="pT", name=f"pT{q}_{pr}")
            for s in range(2):
                nc.tensor.matmul(pT[:, s, :], k_dr[:, pr, s, :, :],
                                 q_dr[:, :, isl], start=True, stop=True,
                                 perf_mode=DR)
            et = pet.tile([128, 2, 512], fp8, tag="et", name=f"et{q}_{pr}")
            nc.scalar.activation(et[:], pT[:], AF.Exp, bias=ebias[:, 0:1],
                                 scale=0.125)
            if dacc_t is not None:
                eng = nc.gpsimd
                if pr == 0:
                    eng.tensor_copy(dacc_t[:], et[:, 0, :])
                else:
                    eng.tensor_add(dacc_t[:], dacc_t[:], et[:, 0, :])
                eng.tensor_add(dacc_t[:], dacc_t[:], et[:, 1, :])
            ets[(q, pr)] = (et, pD)

        def u_stage(q, pr, pu, pD=None):
            et, _ = ets.pop((q, pr))
            for t in range(CT):
                nc.tensor.matmul(pu[t][:], vt[:, t, pr // 2, pr % 2, :, :],
                                 et[:], start=(pr == 0), stop=(pr == NP - 1),
                                 perf_mode=DR)
            if pD is not None:
                nc.tensor.matmul(pD[:], ones8[:], et[:],
                                 start=(pr == 0), stop=(pr == NP - 1),
                                 perf_mode=DR)

        def finish_quarter(q, pu, psD, dacc_t, pD=None):
            isl = slice(q * 512, (q + 1) * 512)
            if dacc_t is not None:
                pD = psD.tile([32, 512], f32, tag="pB", name=f"pDc{q}")
                nc.tensor.matmul(pD[0:1, :], onec[:], dacc_t[:],
                                 start=True, stop=True)
            nc.vector.reciprocal(dr[:, isl], pD[0:1, :])
            pB = psD.tile([128, 512], f32, tag="pB", name=f"pB{q}")
            nc.tensor.matmul(pB[:], oner[:], dr[:, isl], start=True, stop=True)
            rb = recb[q % 2]
            nc.vector.tensor_copy(rb[:], pB[:])
            for t in range(CT):
                nc.vector.tensor_mul(loc16[:, t, isl], pu[t][:], rb[:])

        po = None

        def emit_fuse_quarter(ic):
            isl = slice(ic * 512, (ic + 1) * 512)
            for mt in range(CT):
                pf = psD.tile([128, 512], f32, tag="pB", name=f"pf{ic}_{mt}")
                nc.tensor.matmul(pf[:], fusl[:, 0 + mt, :], loc16[:, 0, isl],
                                 start=True, stop=False)
                nc.tensor.matmul(pf[:], fusl[:, 2 + mt, :], loc16[:, 1, isl],
                                 start=False, stop=False)
                nc.tensor.matmul(pf[:], fusg[:, 0 + mt, :],
                                 xq[:, 0, ic * 8:(ic + 1) * 8, :],
                                 start=False, stop=False)
                nc.tensor.matmul(pf[:], fusg[:, 2 + mt, :],
                                 xq[:, 1, ic * 8:(ic + 1) * 8, :],
                                 start=False, stop=True)
                ob = po.tile([128, 512], f32, tag="ob", name=f"ob{ic}_{mt}")
                nc.vector.tensor_scalar_add(ob[:], pf[:], sp[:, 1 + mt:2 + mt])
                nc.sync.dma_start(out_d[mt, :, isl], ob[:])

        with tc.tile_pool(name="pet", bufs=24) as pet, \
             tc.tile_pool(name="psT", bufs=2, space="PSUM") as psT, \
             tc.tile_pool(name="psU", bufs=1, space="PSUM") as psU:
            pu0 = [psU.tile([128, 512], f32, tag=f"pu{t}", name=f"pu{t}_0")
                   for t in range(CT)]
            def emit_se(psSE):
                def se_psum(name):
                    t = psSE.tile([128, 2, 512], f32, tag="pT", name=name)
                    return t
                # ---- SE channel sums on ACT (accum_out); scratch fp8 output
                sa = pw.tile([128, 1], f32)
                sb_ = pw.tile([128, 1], f32)
                scr = pw.tile([128, 32, 64], fp8, tag="sescr")
                for j in range(CT):
                    nc.scalar.activation(scr[:], xp8[:, j, 1:33, 1:65],
                                         AF.Copy, accum_out=sa[:])
                    nc.scalar.activation(scr[:], xp8[:, j, 35:67, 1:65],
                                         AF.Copy, accum_out=sb_[:])
                    nc.vector.tensor_add(sums[j][:], sa[:], sb_[:])
                # ---- SE MLP: yse = sigmoid(fc2 @ relu(fc1 @ mean + b1) + b2)
                ps1 = se_psum("ps1")
                for j in range(CT):
                    nc.tensor.matmul(ps1[0:16, 0, 0:1],
                                     sp[:, 7 + j * 16:7 + (j + 1) * 16],
                                     sums[j][:], start=(j == 0),
                                     stop=(j == CT - 1))
                y1 = pw.tile([16, 1], f32)
                nc.scalar.activation(y1[:], ps1[0:16, 0, 0:1], AF.Relu,
                                     bias=sq[0:16, 256:257])
                for t in range(CT):
                    ps2 = se_psum(f"ps2_{t}")
                    nc.tensor.matmul(ps2[:, 0, 0:1],
                                     sq[0:16, t * 128:(t + 1) * 128],
                                     y1[:], start=True, stop=True)
                    # sigmoid(z) = 1/(1+exp(-z)); sp col 5+t holds -fc2_b
                    en = pw.tile([128, 1], f32, tag="en")
                    nc.scalar.activation(en[:], ps2[:, 0, 0:1], AF.Exp,
                                         bias=sp[:, 5 + t:6 + t], scale=-1.0)
                    nc.vector.tensor_scalar_add(en[:], en[:], 1.0)
                    nc.vector.reciprocal(yse[t][:], en[:])
            with tc.tile_pool(name="psC", bufs=2, space="PSUM") as psC:
                # ---- conv chunks; quarter-0 attention software-pipelined in
                # (scores lag 2 chunks, U lags 3, so PE never waits on the
                # qk8->k_dr DMA chain or on ACT's exp)
                for c, base in CHUNKS:
                    csl = slice(c * 512, (c + 1) * 512)
                    # qk conv (bf16, 18 passes)
                    pqk = psC.tile([128, 512], f32, tag="pc", name=f"pqk{c}")
                    for t in range(9):
                        for j in range(CT):
                            nc.tensor.matmul(
                                pqk[:], wqk[:, t, j, :],
                                xp16[:, j, base + t // 3:base + t // 3 + 8,
                                     t % 3:t % 3 + 64],
                                start=(t == 0 and j == 0),
                                stop=(t == 8 and j == CT - 1))
                    nc.vector.tensor_scalar_add(qk8[:, csl], pqk[:], sp[:, 0:1])
                    if c == 0:
                        for h in range(2):
                            nc.sync.dma_start(q_dr[:, h, 0:512],
                                              qk8[32 * h:32 * h + 32, 0:512])
                    if c % 2 == 1:
                        p0, p1 = 2 * (c - 1), 2 * (c - 1) + 4
                        hsl = slice(p0 * 256, p1 * 256)
                        for h in range(2):
                            if c == 3:
                                nc.sync.dma_start(
                                    q_dr[:, h, 512:2048],
                                    qk8[32 * h:32 * h + 32, 512:2048])
                            nc.sync.dma_start(
                                k_dr[:, p0:p1, :, h, :],
                                qk8[64 + 32 * h:96 + 32 * h, hsl].rearrange(
                                    "p (pr s j) -> p pr s j", s=2, j=128))
                    # v conv (fp8 DR, 9 passes per ct) -> bf16 -> xbar transpose
                    for ct in range(CT):
                        pv = psC.tile([128, 512], f32, tag="pc", name=f"pv{c}_{ct}")
                        for t in range(9):
                            nc.tensor.matmul(
                                pv[:], wv[:, t, ct, :, :],
                                xp8[:, :, base + t // 3:base + t // 3 + 8,
                                    t % 3:t % 3 + 64],
                                start=(t == 0), stop=(t == 8), perf_mode=DR)
                        nc.vector.tensor_scalar_mul(vsb[ct][:, csl], pv[:],
                                                    1.0 / WSCALE)
                        nc.sync.dma_start_transpose(
                            vt16[:, ct, c, :, :, :], vsb[ct][:, csl])
                        nc.gpsimd.tensor_copy(vt[:, ct, c, :, :, :],
                                              vt16[:, ct, c, :, :, :])
                    if c == 0:
                        emit_se(psT)
                    if c == 4:
                        nc.sync.dma_start(xq[:], xq_d[:])
                        nc.sync.dma_start(fusg[:], fusg_d[:])
                        nc.sync.dma_start(fusl[:], fusl_d[:])

                    if c >= 2:
                        for pl in range(2):
                            sc_stage(0, 2 * (c - 2) + pl, dacc[0], psT, pet)
                    if c >= 4:
                        for pl in range(2):
                            sc_stage(1, 2 * (c - 4) + pl, None, psT, pet, None)
                    if c == 7:
                        for pl in range(2):
                            sc_stage(3, pl, None, psT, pet, None)
                    if c >= 5:
                        for pl in range(2):
                            sc_stage(2, 2 * (c - 5) + pl, None, psT, pet, None)
                    if c >= 3:
                        for pl in range(2):
                            u_stage(0, 2 * (c - 3) + pl, pu0)

            with tc.tile_pool(name="psD", bufs=1, space="PSUM") as psD, \
                 tc.tile_pool(name="po2", bufs=3) as po2:
                po = po2
                # fold yse into glob fuse weights (Pool; after fusg DMA)
                for ct in range(CT):
                    nc.gpsimd.tensor_scalar_mul(fusg[:, 2 * ct:2 * ct + 2, :],
                                                fusg[:, 2 * ct:2 * ct + 2, :],
                                                yse[ct][:, 0:1])
                # flat pipeline over the remaining 56 sc / 58 u stages with
                # inline quarter finishes and per-quarter fuse
                all_sc = [(0, p) for p in range(12, 16)] + \
                         [(1, p) for p in range(8, 16)] + \
                         [(2, p) for p in range(6, 16)] + \
                         [(3, p) for p in range(2, NP)]
                all_u = [(0, p) for p in range(10, 16)] + \
                        [(1, p) for p in range(16)] + \
                        [(q, p) for q in range(2, NQ) for p in range(NP)]
                pus = {0: pu0}
                pDs = {}
                state = {"q": 0}
                pending = []
                for k in range(len(all_u) + 6):
                    if k < len(all_sc):
                        q, p = all_sc[k]
                        sc_stage(q, p, None if q > 0 else dacc[0], psT, pet,
                                 None)
                    if k < len(all_u):
                        q, p = all_u[k]
                        if q > 0 and p == 0:
                            pus[q] = [psU.tile([128, 512], f32, tag=f"pu{t}",
                                               name=f"pu{t}_{q}")
                                      for t in range(CT)]
                            pDs[q] = psD.tile([32, 512], f32, tag="pD",
                                              name=f"pD{q}")
                        u_stage(q, p, pus[q], pDs.get(q))
                        if p == NP - 1:
                            pending.append((k + 6, q))
                    while pending and pending[0][0] <= k:
                        _, fq = pending.pop(0)
                        finish_quarter(fq, pus.pop(fq), psD,
                                       dacc[0] if fq == 0 else None,
                                       pDs.pop(fq, None))
                        emit_fuse_quarter(fq)

        with tc.tile_pool(name="podbg", bufs=1) as podbg:
            if debug:
                dbg_f = pw.tile([128, HW], f32, tag="dbgf")
                nc.vector.tensor_copy(dbg_f[:], qk8[:])
                nc.sync.dma_start(qk8_dbg[:], dbg_f[:])
                dbg_v = pw.tile([128, NP * 512], f32, tag="dbgv")
                nc.vector.tensor_copy(
                    dbg_v[:], vt[:].rearrange("p a b c d e -> p (a b c d e)"))
                nc.sync.dma_start(vt_dbg[:], dbg_v[:])
                dbg_d = pw.tile([1, HWh], f32, tag="dbgd")
                nc.vector.reciprocal(dbg_d[:], dr[:])
                nc.sync.dma_start(d_dbg[:], dbg_d[:])
                dbg_l = pw.tile([128, 2, HWh], f32, tag="dbgl")
                nc.vector.tensor_copy(dbg_l[:], loc16[:])
                nc.sync.dma_start(loc_dbg[:], dbg_l[:])

    nc.compile()
    return nc


def _prep_core_inputs(inputs):
    x = np.ascontiguousarray(inputs["x"], np.float32)
    wq = np.asarray(inputs["wq"], np.float32)
    bq = np.asarray(inputs["bq"], np.float32)
    wk = np.asarray(inputs["wk"], np.float32)
    bk = np.asarray(inputs["bk"], np.float32)
    wv_ = np.asarray(inputs["wv"], np.float32)
    bv = np.asarray(inputs["bv"], np.float32)
    fc1_w = np.asarray(inputs["fc1_w"], np.float32)
    fc1_b = np.asarray(inputs["fc1_b"], np.float32)
    fc2_w = np.asarray(inputs["fc2_w"], np.float32)
    fc2_b = np.asarray(inputs["fc2_b"], np.float32)
    fuse_w = np.asarray(inputs["fuse_w"], np.float32)[:, :, 0, 0]
    fuse_b = np.asarray(inputs["fuse_b"], np.float32)

    wqk = np.concatenate([wq, wk], axis=0)               # [128, 256, 3, 3]
    bqk = np.concatenate([bq, bk])[:, None].astype(np.float32)

    wqk16 = np.empty((128, 9, 2, 128), np.float32)
    wv8 = np.empty((128, 9, 2, 2, 128), np.float32)
    for t in range(9):
        dy, dx = t // 3, t % 3
        for j in range(CT):
            wqk16[:, t, j, :] = wqk[:, j * 128:(j + 1) * 128, dy, dx].T
            for cto in range(CT):
                wv8[:, t, cto, j, :] = (
                    wv_[cto * 128:(cto + 1) * 128, j * 128:(j + 1) * 128,
                        dy, dx].T * WSCALE)

    fusg = np.empty((128, 4, 128), np.float32)
    fusl = np.empty((128, 4, 128), np.float32)
    for ct in range(CT):
        for mt in range(CT):
            fusl[:, 2 * ct + mt, :] = fuse_w[mt * 128:(mt + 1) * 128,
                                             ct * 128:(ct + 1) * 128].T
            fusg[:, 2 * ct + mt, :] = fuse_w[mt * 128:(mt + 1) * 128,
                                             C + ct * 128:C + (ct + 1) * 128].T

    fuse_b_eff = fuse_b + fuse_w[:, :C] @ bv

    smallp = np.zeros((128, 39), np.float32)
    smallp[:, 0:1] = bqk
    smallp[:, 1:3] = np.stack([fuse_b_eff[t * 128:(t + 1) * 128]
                               for t in range(CT)], axis=1)
    smallp[:, 5:7] = np.stack([-fc2_b[t * 128:(t + 1) * 128]
                               for t in range(CT)], axis=1)
    for j in range(CT):
        smallp[:, 7 + j * 16:7 + (j + 1) * 16] = (fc1_w / HW)[:, j * 128:(j + 1) * 128].T
    smallq = np.zeros((16, 257), np.float32)
    for t in range(CT):
        smallq[:, t * 128:(t + 1) * 128] = fc2_w[t * 128:(t + 1) * 128, :].T
    smallq[:, 256] = fc1_b

    shared = dict(
        wqk=np.ascontiguousarray(wqk16).astype(b16np),
        wv=np.ascontiguousarray(wv8).astype(f8np),
        fusg=np.ascontiguousarray(fusg).astype(b16np),
        fusl=np.ascontiguousarray(fusl).astype(b16np),
        smallp=smallp, smallq=smallq,
    )

    in_maps = []
    for core in range(N_CORES):
        s, p = divmod(core, 2)
        s0 = p * 32
        t0 = (s0 + 32) % 64
        P = np.zeros((C, 66, 66), np.float32)
        P[:, 1:65, 1:65] = x[s]
        xp = np.concatenate([P[:, s0:s0 + 34], P[:, t0:t0 + 34]], axis=1)
        xp4 = xp.reshape(2, 128, 68, 66).transpose(1, 0, 2, 3)
        xqc = xp[:, 1:33, 1:65].reshape(2, 128, 32, 64).transpose(1, 0, 2, 3)
        m = dict(shared)
        m["xp16"] = np.ascontiguousarray(xp4).astype(b16np)
        m["xp8"] = np.ascontiguousarray(xp4).astype(f8np)
        m["xq"] = np.ascontiguousarray(xqc).astype(b16np)
        in_maps.append(m)
    return in_maps


def kernel(**inputs):
    global _compiled
    if _compiled is None:
        _compiled = _build()
    nc = _compiled
    in_maps = _prep_core_inputs(inputs)
    res = run_bass_kernel_spmd(nc, in_maps, list(range(N_CORES)))
    out = np.empty((4, C, H, W), np.float32)
    for core in range(N_CORES):
        s, p = divmod(core, 2)
        o = res.results[core]["out"]          # [2, 128, 2048]
        out[s, :, p * 32:(p + 1) * 32, :] = o.reshape(C, 32, 64)
    return out
